# revision 16
# baseline (speedup 1.0000x reference)
"""Causal self-attention (B=2, S=2048, E=1024, H=16, D=64) on 8 trn2 NeuronCores.

Sharding: core c = (batch b = c // 4, head-group g = c % 4).  Each core computes
4 heads (one quarter of the 16) for one batch: projections q/k/v for its 256
output channels, then causal flash-style attention over head pairs.

Per-core kernel design (Bass/Tile):
  - Host pre-transposes hidden -> hT [E, S] (bf16) and weight slices -> wT [E, 256]
    (bf16) so all matmul contractions have K on partitions.
  - q/k projections (bf16, PSUM-accumulated over 8 E-chunks) produce qT/kT in
    [d, t] layout (bf16) with scale 1/8 (q) and bias add.
  - v projection produces v in [t, d] layout; DVE copy splits heads into
    v_aug tiles [tk=128, 65*2] with a ones column per head (sum-of-exp trick).
  - scores^T tiles [tk=128, tq<=512] per head via row-tiled bf16 matmuls (the
    two heads of a pair run concurrently on PE row halves, K=64 each).
  - exp via ScalarE activation (mask-2.0 bias per tk partition), bf16 out.
  - causal masking: gpsimd affine_select zeroes the above-diagonal triangle of
    the 128-col diagonal band of crossing tiles after exp.
  - attn @ v_aug accumulates unnormalized out^T [65, tq] in PSUM (bf16 matmuls);
    row 64 is the softmax denominator.  All (block, tile) tasks run through one
    flat software pipeline: scores/exp of tile t issue ahead of the attn@v of
    tile t-LAG so the in-order PE queue never stalls on exp latency.
  - DVE evacuates [65, 512] f32 tiles; DMA to DRAM; the host divides by the
    denominator row and transposes into [t, h*d].
"""

import numpy as np
import ml_dtypes

import concourse.bass as bass
import concourse.mybir as mybir
import concourse.tile as tile
from concourse import bacc
from concourse.bass_utils import run_bass_kernel_spmd

F32 = mybir.dt.float32
BF16 = mybir.dt.bfloat16

B, S, E = 2, 2048, 1024
H, D = 16, 64
NCORES = 8
OC = 256          # output channels per core (4 heads)
NPAIR = 2         # head pairs per core
NT = S // 128     # 16 tk tiles
NT4 = S // 512    # 4 tq blocks
SHIFT = 2.0       # subtracted from scores pre-exp (via mask bias); cancels in
                  # normalization
LAG = 2           # attn@v trails scores/exp by this many pipeline slots
N_WARM = 56       # dummy matmuls during the DMA wait keep the PE HAM clock
                  # gate warm so real chains start at 2.4 GHz
# Schraudolph fast-exp constants: exp(x) ~= bitcast_f32(int32(A*x + B))
SCH_A = 12102203.16  # 2^23 / ln 2
SCH_B = float(127 * (1 << 23) - 486411)
# late-phase tiles whose exp runs on DVE (Schraudolph) instead of the
# saturated ScalarE: flat pipeline indices (back half, alternating; denser in
# the chore-free tail where ACT is otherwise the serial wall)
SCH_TILES = frozenset((45, 47, 53, 55, 57, 59, 63, 65, 67, 69, 71, 73, 75, 77))

_cached_nc = None


def _patch_ldw_opt():
    # walrus is invoked with --enable-ldw-opt=false hardcoded; LDWEIGHTS
    # scheduling opt measurably tightens back-to-back matmul spacing.
    import os
    if os.environ.get("LDW_OPT", "0") != "1":
        return
    import concourse.bass_utils as _bu
    if getattr(_bu, "_ldw_patched", False):
        return
    _orig = _bu.run_command

    def _patched(argv, **kw):
        argv = ["--enable-ldw-opt=true" if a == "--enable-ldw-opt=false" else a
                for a in argv]
        return _orig(argv, **kw)

    _bu.run_command = _patched
    _bu._ldw_patched = True


def _build():
    _patch_ldw_opt()
    nc = bacc.Bacc()

    hT = nc.declare_dram_parameter("hT", [128, 32 * 512], BF16, isOutput=False)
    wqT = nc.declare_dram_parameter("wqT", [128, 2048], BF16, isOutput=False)
    wkT = nc.declare_dram_parameter("wkT", [128, 2048], BF16, isOutput=False)
    wvT = nc.declare_dram_parameter("wvT", [128, 2048], BF16, isOutput=False)
    bqp = nc.declare_dram_parameter("bqp", [128, 2], F32, isOutput=False)
    bkp = nc.declare_dram_parameter("bkp", [128, 2], F32, isOutput=False)
    bvf = nc.declare_dram_parameter("bvf", [OC], F32, isOutput=False)
    mask_t = nc.declare_dram_parameter("mask_t", [128, NT], F32, isOutput=False)
    out = nc.declare_dram_parameter("out", [4 * 65, S], F32, isOutput=True)

    EXP = mybir.ActivationFunctionType.Exp
    ADD = mybir.AluOpType.add
    MULT = mybir.AluOpType.mult
    GE = mybir.AluOpType.is_ge
    I32 = mybir.dt.int32

    with tile.TileContext(nc) as tc:
        with (
            tc.tile_pool(name="cst", bufs=1) as cst,
            tc.tile_pool(name="work", bufs=4) as work,
            tc.tile_pool(name="expp", bufs=6) as expp,
            tc.tile_pool(name="ps_small", bufs=2, space="PSUM") as ps_small,
            tc.tile_pool(name="ps_sc", bufs=2, space="PSUM") as ps_sc,
            tc.tile_pool(name="ps_out", bufs=2, space="PSUM") as ps_out,
        ):
            # ---- PE warmup: dummy matmuls during the DMA wait (no DMA deps)
            # keep the HAM activity window busy so real chains start warm ----
            dum = cst.tile([128, 64], BF16, tag="dum")
            nc.vector.memset(dum, 0.5)
            for _w in range(N_WARM):
                ps_d = ps_small.tile([128, 64], F32, tag="sm", name="warm")
                nc.tensor.matmul(ps_d[0:64, :], dum, dum, start=True, stop=True)

            # ---- big resident inputs, host-packed in consumption order:
            # wq/wk pair-major (pair p at cols 1024p) so the first chains need
            # only the pair-0 piece; hT in t4 groups 3,0,1,2. ----
            G = {3: 0, 0: 1, 1: 2, 2: 3}  # t4 -> group position
            hT_big = cst.tile([128, 32 * 512], BF16, tag="hT_big")
            wq_big = cst.tile([128, 2048], BF16, tag="wq_big")
            wk_big = cst.tile([128, 2048], BF16, tag="wk_big")
            wv_big = cst.tile([128, 2048], BF16, tag="wv_big")
            # first critical pieces go out on idle engine queues — the sync
            # queue is busy with semaphore setup for several us at kernel start
            nc.scalar.dma_start(out=wq_big[:, 0:1024], in_=wqT[:, 0:1024])
            nc.scalar.dma_start(out=hT_big[:, 0:2048], in_=hT[:, 0:2048])
            nc.gpsimd.dma_start(out=hT_big[:, 2048:4096], in_=hT[:, 2048:4096])
            nc.scalar.dma_start(out=wk_big[:, 0:1024], in_=wkT[:, 0:1024])
            bq_sb = cst.tile([128, 2], F32, tag="bq")
            nc.gpsimd.dma_start(out=bq_sb, in_=bqp[:, :])
            bk_sb = cst.tile([128, 2], F32, tag="bk")
            nc.gpsimd.dma_start(out=bk_sb, in_=bkp[:, :])
            mask_sb = cst.tile([128, NT], F32, tag="mask")
            nc.gpsimd.dma_start(out=mask_sb, in_=mask_t[:, :])
            nc.sync.dma_start(out=hT_big[:, 4096:6144], in_=hT[:, 4096:6144])
            nc.sync.dma_start(out=hT_big[:, 6144:8192], in_=hT[:, 6144:8192])
            nc.sync.dma_start(out=wv_big[:, 0:1024], in_=wvT[:, 0:1024])
            nc.sync.dma_start(out=wv_big[:, 1024:2048], in_=wvT[:, 1024:2048])
            bv_sb = cst.tile([128, OC], F32, tag="bv")
            nc.gpsimd.dma_start(out=bv_sb, in_=bvf[:].partition_broadcast(128))
            nc.sync.dma_start(out=hT_big[:, 8192:12288], in_=hT[:, 8192:12288])
            nc.sync.dma_start(out=wq_big[:, 1024:2048], in_=wqT[:, 1024:2048])
            nc.sync.dma_start(out=wk_big[:, 1024:2048], in_=wkT[:, 1024:2048])
            nc.sync.dma_start(out=hT_big[:, 12288:16384], in_=hT[:, 12288:16384])

            # Schraudolph per-partition affine term: A*mask' + B
            amb = cst.tile([128, NT], F32, tag="amb")
            nc.vector.tensor_scalar(out=amb, in0=mask_sb, scalar1=SCH_A,
                                    scalar2=SCH_B, op0=MULT, op1=ADD)

            hT32 = [[hT_big[:, G[t4] * 4096 + e * 512: G[t4] * 4096 + (e + 1) * 512]
                     for t4 in range(NT4)] for e in range(8)]
            # wq/wk pair-major: [pair p][e-chunk] at cols 1024p + 128e
            wqk_sb = {nm: [[big[:, 1024 * p + 128 * e: 1024 * p + 128 * (e + 1)]
                            for e in range(8)] for p in range(NPAIR)]
                      for nm, big in (("q", wq_big), ("k", wk_big))}
            wv_sb = [wv_big[:, e * OC:(e + 1) * OC] for e in range(8)]

            # ---- persistent intermediates ----
            qT = [cst.tile([128, S], BF16, tag=f"qT{p}", name=f"qT{p}") for p in range(NPAIR)]
            kT = [cst.tile([128, S], BF16, tag=f"kT{p}", name=f"kT{p}") for p in range(NPAIR)]
            vaug = [[cst.tile([128, 256], BF16, tag=f"va{p}_{tt}", name=f"va{p}_{tt}")
                     for tt in range(NT)] for p in range(NPAIR)]

            def emit_qk_chain(nm, p, t4):
                dst = qT[p] if nm == "q" else kT[p]
                b_sb = bq_sb if nm == "q" else bk_sb
                ts = slice(512 * t4, 512 * (t4 + 1))
                ps_qk = ps_small.tile([128, 512], F32, tag="sm", name="ps_qk")
                for e in range(8):
                    nc.tensor.matmul(
                        ps_qk,
                        wqk_sb[nm][p][e],
                        hT32[e][t4],
                        start=(e == 0), stop=(e == 7),
                    )
                if nm == "q":
                    nc.vector.tensor_scalar(
                        out=dst[:, ts], in0=ps_qk,
                        scalar1=0.125, scalar2=b_sb[:, p:p + 1],
                        op0=MULT, op1=ADD,
                    )
                else:
                    nc.vector.tensor_scalar_add(
                        out=dst[:, ts], in0=ps_qk, scalar1=b_sb[:, p:p + 1],
                    )

            def emit_v_chain(tt):
                t4v, r4 = divmod(tt, 4)
                rs = slice(128 * r4, 128 * (r4 + 1))
                ps_v = ps_small.tile([128, OC], F32, tag="sm", name="ps_v")
                for e in range(8):
                    nc.tensor.matmul(
                        ps_v,
                        hT32[e][t4v][:, rs],
                        wv_sb[e],
                        start=(e == 0), stop=(e == 7),
                    )
                for p in range(NPAIR):
                    po = 128 * p
                    vt = vaug[p][tt]
                    vt3 = vt.rearrange("a (h c) -> a h c", h=2)[:, :, 0:64]
                    ps3 = ps_v[:, po:po + 128].rearrange("a (h c) -> a h c", h=2)
                    bv3 = bv_sb[:, po:po + 128].rearrange("a (h c) -> a h c", h=2)
                    nc.vector.tensor_add(vt3, ps3, bv3)
                    # ones column for the sum-of-exp denominator; cols 65:128
                    # stay uninitialized (their psum rows are never read)
                    nc.vector.memset(
                        vt.rearrange("a (h c) -> a h c", h=2)[:, :, 64:65], 1.0)

            chores_q = []

            # ---- flat attention pipeline across all (p, j) blocks ----
            blocks = [(0, 3), (0, 2), (1, 3), (0, 1), (1, 2), (0, 0), (1, 1), (1, 0)]
            tasks = []  # (p, j, i, ntk)
            for p, j in blocks:
                ntk = 4 * (j + 1)
                for i in range(ntk):
                    tasks.append((p, j, i, ntk))
            bstate = {}  # (p, j) -> (out_A, out_B)
            exs = {}     # flat index -> ex tile

            def emit_scores_exp(t):
                p, j, i, ntk = tasks[t]
                crossing = i >= 4 * j
                s = 128 * i - 512 * j if crossing else 0
                ks = slice(128 * i, 128 * (i + 1))
                qsv = slice(512 * j + s, 512 * (j + 1))
                sc = ps_sc.tile([128, 1024], F32, tag="sc", name="sc")
                nc.tensor.matmul(sc[:, s:512], kT[p][0:64, ks],
                                 qT[p][0:64, qsv], start=True, stop=True)
                nc.tensor.matmul(sc[:, 512 + s:1024], kT[p][64:128, ks],
                                 qT[p][64:128, qsv], start=True, stop=True)
                ex = expp.tile([128, 1024], BF16, tag="exp", name="ex")
                if s:
                    exv = ex.rearrange("a (h f) -> a h f", h=2)[:, :, s:512]
                    scv = sc.rearrange("a (h f) -> a h f", h=2)[:, :, s:512]
                else:
                    exv, scv = ex, sc
                if t in SCH_TILES:
                    # Schraudolph fast exp on DVE (ScalarE is saturated late):
                    # ex = bitcast_f32(int32(A*(score) + (A*mask' + B)))
                    i32t = work.tile([128, 1024], I32, tag="sch", name="sch")
                    if s:
                        i32v = i32t.rearrange("a (h f) -> a h f", h=2)[:, :, s:512]
                    else:
                        i32v = i32t
                    nc.vector.tensor_scalar(
                        out=i32v, in0=scv, scalar1=SCH_A,
                        scalar2=amb[:, i:i + 1], op0=MULT, op1=ADD)
                    nc.vector.tensor_copy(exv, i32v.bitcast(F32))
                else:
                    nc.scalar.activation(out=exv, in_=scv, func=EXP,
                                         bias=mask_sb[:, i:i + 1], scale=1.0)
                if crossing:
                    # zero the above-diagonal triangle in the 128-col band
                    ex3 = ex.rearrange("a (h f) -> a h f", h=2)[:, :, s:s + 128]
                    nc.gpsimd.affine_select(
                        out=ex3, in_=ex3, compare_op=GE, fill=0.0,
                        base=0, pattern=[[0, 2], [1, 128]],
                        channel_multiplier=-1,
                    )
                exs[t] = ex

            def emit_av(t):
                p, j, i, ntk = tasks[t]
                crossing = i >= 4 * j
                s = 128 * i - 512 * j if crossing else 0
                if i == 0:
                    out_A = ps_out.tile([128, 512], F32, tag="out", name="out_A")
                    out_B = ps_out.tile([128, 512], F32, tag="out", name="out_B")
                    bstate[(p, j)] = (out_A, out_B)
                out_A, out_B = bstate[(p, j)]
                ex = exs.pop(t)
                nc.tensor.matmul(out_A[:, s:512], vaug[p][i][:, 0:128],
                                 ex[:, s:512],
                                 start=(i == 0), stop=(i == ntk - 1))
                nc.tensor.matmul(out_B[:, s:512], vaug[p][i][:, 128:256],
                                 ex[:, 512 + s:1024],
                                 start=(i == 0), stop=(i == ntk - 1))
                if i == ntk - 1:
                    tail = t >= 60  # chore-free endgame: split evac ACT/DVE
                    for h_loc, out_ps in ((0, out_A), (1, out_B)):
                        osb = work.tile([65, 512], F32, tag="u", name="osb")
                        if tail and h_loc == 0:
                            nc.scalar.activation(
                                out=osb, in_=out_ps[0:65, :],
                                func=mybir.ActivationFunctionType.Copy)
                        else:
                            nc.vector.tensor_copy(osb, out_ps[0:65, :])
                        r0 = 130 * p + 65 * h_loc
                        nc.sync.dma_start(
                            out=out[r0:r0 + 65, 512 * j:512 * (j + 1)], in_=osb)

            # prologue: minimal producers for attention(p0, j=3, i=0..)
            emit_qk_chain("q", 0, 3)
            emit_qk_chain("k", 0, 0)
            for tt in range(4):
                emit_v_chain(tt)
            chores_q.append(lambda: emit_qk_chain("k", 0, 1))
            chores_q.append(lambda: emit_v_chain(4))
            chores_q.append(lambda: emit_v_chain(5))
            chores_q.append(lambda: emit_v_chain(6))
            chores_q.append(lambda: emit_qk_chain("k", 0, 2))
            chores_q.append(lambda: emit_v_chain(7))
            chores_q.append(lambda: emit_v_chain(8))
            chores_q.append(lambda: emit_v_chain(9))
            chores_q.append(lambda: emit_qk_chain("k", 0, 3))
            for _tt in range(10, 16):
                chores_q.append(lambda _tt=_tt: emit_v_chain(_tt))
            chores_q.append(lambda: emit_qk_chain("q", 0, 2))
            chores_q.append(lambda: emit_qk_chain("q", 0, 1))
            chores_q.append(lambda: emit_qk_chain("q", 0, 0))
            for t4 in (3, 2, 1, 0):
                chores_q.append(lambda t4=t4: emit_qk_chain("q", 1, t4))
                chores_q.append(lambda t4=t4: emit_qk_chain("k", 1, 3 - t4))

            ntasks = len(tasks)
            for t in range(ntasks + LAG):
                if t < ntasks:
                    emit_scores_exp(t)
                if t - LAG >= 0:
                    emit_av(t - LAG)
                if chores_q:
                    chores_q.pop(0)()

    nc.compile()
    return nc


def _get_nc():
    global _cached_nc
    if _cached_nc is None:
        _cached_nc = _build()
    return _cached_nc


def make_in_maps(hidden_states, attention_mask, Wq, bq, Wk, bk, Wv, bv):
    hidden_states = np.asarray(hidden_states, dtype=np.float32)
    attention_mask = np.asarray(attention_mask, dtype=np.float32)
    Wq = np.asarray(Wq, dtype=np.float32)
    Wk = np.asarray(Wk, dtype=np.float32)
    Wv = np.asarray(Wv, dtype=np.float32)
    bq = np.asarray(bq, dtype=np.float32)
    bk = np.asarray(bk, dtype=np.float32)
    bv = np.asarray(bv, dtype=np.float32)

    bf = ml_dtypes.bfloat16
    in_maps = []
    for c in range(NCORES):
        b, g = divmod(c, 4)
        cs = slice(OC * g, OC * (g + 1))
        hTT = np.ascontiguousarray(hidden_states[b].T).astype(bf)  # [E, S]
        hp = np.empty((128, 32 * 512), dtype=bf)
        for gi, t4 in enumerate((3, 0, 1, 2)):
            for e in range(8):
                hp[:, gi * 4096 + e * 512:gi * 4096 + (e + 1) * 512] = \
                    hTT[e * 128:(e + 1) * 128, t4 * 512:(t4 + 1) * 512]

        def packw(W):
            # e-major: [e-chunk e][all 256 channels] at cols 256e (v path)
            wT = np.ascontiguousarray(W[cs, :].T).astype(bf)  # [E, 256]
            wp = np.empty((128, 2048), dtype=bf)
            for e in range(8):
                wp[:, e * OC:(e + 1) * OC] = wT[e * 128:(e + 1) * 128, :]
            return wp

        def packw_pair(W):
            # pair-major: pair p at cols 1024p, e-chunk e at 128e within
            wT = np.ascontiguousarray(W[cs, :].T).astype(bf)  # [E, 256]
            wp = np.empty((128, 2048), dtype=bf)
            for p in range(2):
                for e in range(8):
                    wp[:, 1024 * p + 128 * e:1024 * p + 128 * (e + 1)] = \
                        wT[e * 128:(e + 1) * 128, 128 * p:128 * (p + 1)]
            return wp

        in_maps.append({
            "hT": hp,
            "wqT": packw_pair(Wq),
            "wkT": packw_pair(Wk),
            "wvT": packw(Wv),
            "bqp": np.ascontiguousarray(bq[cs].reshape(2, 128).T),
            "bkp": np.ascontiguousarray(bk[cs].reshape(2, 128).T),
            "bvf": np.ascontiguousarray(bv[cs]),
            "mask_t": np.ascontiguousarray(
                (attention_mask[b, 0, 0, :] - SHIFT).reshape(NT, 128).T),
        })
    return in_maps


def kernel(hidden_states, attention_mask, Wq, bq, Wk, bk, Wv, bv):
    in_maps = make_in_maps(hidden_states, attention_mask,
                           Wq, bq, Wk, bk, Wv, bv)
    nc = _get_nc()
    res = run_bass_kernel_spmd(nc, in_maps, list(range(NCORES)))

    full = np.empty((B, S, H * D), dtype=np.float32)
    for c in range(NCORES):
        b, g = divmod(c, 4)
        r = res.results[c]["out"]  # [260, S] unnormalized out^T + denom rows
        for p in range(NPAIR):
            for h in range(2):
                blk = r[130 * p + 65 * h: 130 * p + 65 * h + 65]
                c0 = OC * g + 128 * p + 64 * h
                full[b, :, c0:c0 + 64] = (blk[0:64] / blk[64:65]).T
    return full


# revision 19
# speedup vs baseline: 1.0140x; 1.0140x over previous
"""Causal self-attention (B=2, S=2048, E=1024, H=16, D=64) on 8 trn2 NeuronCores.

Sharding: core c = (batch b = c // 4, head-group g = c % 4).  Each core computes
4 heads (one quarter of the 16) for one batch: projections q/k/v for its 256
output channels, then causal flash-style attention over head pairs.

Per-core kernel design (Bass/Tile):
  - Host pre-transposes hidden -> hT [E, S] (bf16) and weight slices -> wT [E, 256]
    (bf16) so all matmul contractions have K on partitions.
  - q/k projections (bf16, PSUM-accumulated over 8 E-chunks) produce qT/kT in
    [d, t] layout (bf16) with scale 1/8 (q) and bias add.
  - v projection produces v in [t, d] layout; DVE copy splits heads into
    v_aug tiles [tk=128, 65*2] with a ones column per head (sum-of-exp trick).
  - scores^T tiles [tk=128, tq<=512] per head via row-tiled bf16 matmuls (the
    two heads of a pair run concurrently on PE row halves, K=64 each).
  - exp via ScalarE activation (mask-2.0 bias per tk partition), bf16 out.
  - causal masking: gpsimd affine_select zeroes the above-diagonal triangle of
    the 128-col diagonal band of crossing tiles after exp.
  - attn @ v_aug accumulates unnormalized out^T [65, tq] in PSUM (bf16 matmuls);
    row 64 is the softmax denominator.  All (block, tile) tasks run through one
    flat software pipeline: scores/exp of tile t issue ahead of the attn@v of
    tile t-LAG so the in-order PE queue never stalls on exp latency.
  - DVE evacuates [65, 512] f32 tiles; DMA to DRAM; the host divides by the
    denominator row and transposes into [t, h*d].
"""

import numpy as np
import ml_dtypes

import concourse.bass as bass
import concourse.mybir as mybir
import concourse.tile as tile
from concourse import bacc
from concourse.bass_utils import run_bass_kernel_spmd

F32 = mybir.dt.float32
BF16 = mybir.dt.bfloat16

B, S, E = 2, 2048, 1024
H, D = 16, 64
NCORES = 8
OC = 256          # output channels per core (4 heads)
NPAIR = 2         # head pairs per core
NT = S // 128     # 16 tk tiles
NT4 = S // 512    # 4 tq blocks
SHIFT = 2.0       # subtracted from scores pre-exp (via mask bias); cancels in
                  # normalization
LAG = 2           # attn@v trails scores/exp by this many pipeline slots
N_WARM = 56       # dummy matmuls during the DMA wait keep the PE HAM clock
                  # gate warm so real chains start at 2.4 GHz
# Schraudolph fast-exp, bf16-direct variant: bf16 bits are the top 16 of f32,
# so int16((A*x + B) / 65536) IS the bf16 bit pattern of exp(x) -- one DVE
# tensor_scalar per tile, output written through a bf16<->int16 bitcast view.
SCH_A = 12102203.16 / 65536.0  # (2^23 / ln 2) >> 16
SCH_B = float(127 * (1 << 23) - 486411) / 65536.0
# chore-free back half: alternate exp between ScalarE and DVE so the two
# engines form parallel exp pipes (ACT alone is the serial wall there)
SCH_TILES = frozenset(t for t in range(29, 80) if t % 2 == 1)

_cached_nc = None


def _patch_ldw_opt():
    # walrus is invoked with --enable-ldw-opt=false hardcoded; LDWEIGHTS
    # scheduling opt measurably tightens back-to-back matmul spacing.
    import os
    if os.environ.get("LDW_OPT", "0") != "1":
        return
    import concourse.bass_utils as _bu
    if getattr(_bu, "_ldw_patched", False):
        return
    _orig = _bu.run_command

    def _patched(argv, **kw):
        argv = ["--enable-ldw-opt=true" if a == "--enable-ldw-opt=false" else a
                for a in argv]
        return _orig(argv, **kw)

    _bu.run_command = _patched
    _bu._ldw_patched = True


def _build():
    _patch_ldw_opt()
    nc = bacc.Bacc()

    hT = nc.declare_dram_parameter("hT", [128, 32 * 512], BF16, isOutput=False)
    wqT = nc.declare_dram_parameter("wqT", [128, 2048], BF16, isOutput=False)
    wkT = nc.declare_dram_parameter("wkT", [128, 2048], BF16, isOutput=False)
    wvT = nc.declare_dram_parameter("wvT", [128, 2048], BF16, isOutput=False)
    bqp = nc.declare_dram_parameter("bqp", [128, 2], F32, isOutput=False)
    bkp = nc.declare_dram_parameter("bkp", [128, 2], F32, isOutput=False)
    bvf = nc.declare_dram_parameter("bvf", [OC], F32, isOutput=False)
    mask_t = nc.declare_dram_parameter("mask_t", [128, NT], F32, isOutput=False)
    out = nc.declare_dram_parameter("out", [4 * 65, S], F32, isOutput=True)

    EXP = mybir.ActivationFunctionType.Exp
    ADD = mybir.AluOpType.add
    MULT = mybir.AluOpType.mult
    GE = mybir.AluOpType.is_ge
    I32 = mybir.dt.int32

    with tile.TileContext(nc) as tc:
        with (
            tc.tile_pool(name="cst", bufs=1) as cst,
            tc.tile_pool(name="work", bufs=4) as work,
            tc.tile_pool(name="expp", bufs=6) as expp,
            tc.tile_pool(name="ps_small", bufs=2, space="PSUM") as ps_small,
            tc.tile_pool(name="ps_sc", bufs=2, space="PSUM") as ps_sc,
            tc.tile_pool(name="ps_out", bufs=2, space="PSUM") as ps_out,
        ):
            # ---- PE warmup: dummy matmuls during the DMA wait (no DMA deps)
            # keep the HAM activity window busy so real chains start warm ----
            dum = cst.tile([128, 64], BF16, tag="dum")
            nc.vector.memset(dum, 0.5)
            for _w in range(N_WARM):
                ps_d = ps_small.tile([128, 64], F32, tag="sm", name="warm")
                nc.tensor.matmul(ps_d[0:64, :], dum, dum, start=True, stop=True)

            # ---- big resident inputs, host-packed in consumption order:
            # wq/wk pair-major (pair p at cols 1024p) so the first chains need
            # only the pair-0 piece; hT in t4 groups 3,0,1,2. ----
            G = {3: 0, 0: 1, 1: 2, 2: 3}  # t4 -> group position
            hT_big = cst.tile([128, 32 * 512], BF16, tag="hT_big")
            wq_big = cst.tile([128, 2048], BF16, tag="wq_big")
            wk_big = cst.tile([128, 2048], BF16, tag="wk_big")
            wv_big = cst.tile([128, 2048], BF16, tag="wv_big")
            # first critical pieces go out on idle engine queues — the sync
            # queue is busy with semaphore setup for several us at kernel start
            nc.scalar.dma_start(out=wq_big[:, 0:1024], in_=wqT[:, 0:1024])
            nc.scalar.dma_start(out=hT_big[:, 0:2048], in_=hT[:, 0:2048])
            nc.gpsimd.dma_start(out=hT_big[:, 2048:4096], in_=hT[:, 2048:4096])
            nc.scalar.dma_start(out=wk_big[:, 0:1024], in_=wkT[:, 0:1024])
            bq_sb = cst.tile([128, 2], F32, tag="bq")
            nc.gpsimd.dma_start(out=bq_sb, in_=bqp[:, :])
            bk_sb = cst.tile([128, 2], F32, tag="bk")
            nc.gpsimd.dma_start(out=bk_sb, in_=bkp[:, :])
            mask_sb = cst.tile([128, NT], F32, tag="mask")
            nc.gpsimd.dma_start(out=mask_sb, in_=mask_t[:, :])
            nc.sync.dma_start(out=hT_big[:, 4096:6144], in_=hT[:, 4096:6144])
            nc.sync.dma_start(out=hT_big[:, 6144:8192], in_=hT[:, 6144:8192])
            nc.sync.dma_start(out=wv_big[:, 0:1024], in_=wvT[:, 0:1024])
            nc.sync.dma_start(out=wv_big[:, 1024:2048], in_=wvT[:, 1024:2048])
            bv_sb = cst.tile([128, OC], F32, tag="bv")
            nc.gpsimd.dma_start(out=bv_sb, in_=bvf[:].partition_broadcast(128))
            nc.sync.dma_start(out=hT_big[:, 8192:12288], in_=hT[:, 8192:12288])
            nc.sync.dma_start(out=wq_big[:, 1024:2048], in_=wqT[:, 1024:2048])
            nc.sync.dma_start(out=wk_big[:, 1024:2048], in_=wkT[:, 1024:2048])
            nc.sync.dma_start(out=hT_big[:, 12288:16384], in_=hT[:, 12288:16384])

            # Schraudolph per-partition affine term: A*mask' + B
            amb = cst.tile([128, NT], F32, tag="amb")
            nc.vector.tensor_scalar(out=amb, in0=mask_sb, scalar1=SCH_A,
                                    scalar2=SCH_B, op0=MULT, op1=ADD)

            hT32 = [[hT_big[:, G[t4] * 4096 + e * 512: G[t4] * 4096 + (e + 1) * 512]
                     for t4 in range(NT4)] for e in range(8)]
            # wq/wk pair-major: [pair p][e-chunk] at cols 1024p + 128e
            wqk_sb = {nm: [[big[:, 1024 * p + 128 * e: 1024 * p + 128 * (e + 1)]
                            for e in range(8)] for p in range(NPAIR)]
                      for nm, big in (("q", wq_big), ("k", wk_big))}
            wv_sb = [wv_big[:, e * OC:(e + 1) * OC] for e in range(8)]

            # ---- persistent intermediates ----
            qT = [cst.tile([128, S], BF16, tag=f"qT{p}", name=f"qT{p}") for p in range(NPAIR)]
            kT = [cst.tile([128, S], BF16, tag=f"kT{p}", name=f"kT{p}") for p in range(NPAIR)]
            vaug = [[cst.tile([128, 256], BF16, tag=f"va{p}_{tt}", name=f"va{p}_{tt}")
                     for tt in range(NT)] for p in range(NPAIR)]

            def emit_qk_chain(nm, p, t4):
                dst = qT[p] if nm == "q" else kT[p]
                b_sb = bq_sb if nm == "q" else bk_sb
                ts = slice(512 * t4, 512 * (t4 + 1))
                ps_qk = ps_small.tile([128, 512], F32, tag="sm", name="ps_qk")
                for e in range(8):
                    nc.tensor.matmul(
                        ps_qk,
                        wqk_sb[nm][p][e],
                        hT32[e][t4],
                        start=(e == 0), stop=(e == 7),
                    )
                if nm == "q":
                    nc.vector.tensor_scalar(
                        out=dst[:, ts], in0=ps_qk,
                        scalar1=0.125, scalar2=b_sb[:, p:p + 1],
                        op0=MULT, op1=ADD,
                    )
                else:
                    nc.vector.tensor_scalar_add(
                        out=dst[:, ts], in0=ps_qk, scalar1=b_sb[:, p:p + 1],
                    )

            def emit_v_chain(tt):
                t4v, r4 = divmod(tt, 4)
                rs = slice(128 * r4, 128 * (r4 + 1))
                ps_v = ps_small.tile([128, OC], F32, tag="sm", name="ps_v")
                for e in range(8):
                    nc.tensor.matmul(
                        ps_v,
                        hT32[e][t4v][:, rs],
                        wv_sb[e],
                        start=(e == 0), stop=(e == 7),
                    )
                for p in range(NPAIR):
                    po = 128 * p
                    vt = vaug[p][tt]
                    vt3 = vt.rearrange("a (h c) -> a h c", h=2)[:, :, 0:64]
                    ps3 = ps_v[:, po:po + 128].rearrange("a (h c) -> a h c", h=2)
                    bv3 = bv_sb[:, po:po + 128].rearrange("a (h c) -> a h c", h=2)
                    nc.vector.tensor_add(vt3, ps3, bv3)
                    # ones column for the sum-of-exp denominator; cols 65:128
                    # stay uninitialized (their psum rows are never read)
                    nc.vector.memset(
                        vt.rearrange("a (h c) -> a h c", h=2)[:, :, 64:65], 1.0)

            chores_q = []

            # ---- flat attention pipeline across all (p, j) blocks ----
            blocks = [(0, 3), (0, 2), (1, 3), (0, 1), (1, 2), (0, 0), (1, 1), (1, 0)]
            tasks = []  # (p, j, i, ntk)
            for p, j in blocks:
                ntk = 4 * (j + 1)
                for i in range(ntk):
                    tasks.append((p, j, i, ntk))
            bstate = {}  # (p, j) -> (out_A, out_B)
            exs = {}     # flat index -> ex tile

            def emit_scores_exp(t):
                p, j, i, ntk = tasks[t]
                crossing = i >= 4 * j
                s = 128 * i - 512 * j if crossing else 0
                ks = slice(128 * i, 128 * (i + 1))
                qsv = slice(512 * j + s, 512 * (j + 1))
                sc = ps_sc.tile([128, 1024], F32, tag="sc", name="sc")
                nc.tensor.matmul(sc[:, s:512], kT[p][0:64, ks],
                                 qT[p][0:64, qsv], start=True, stop=True)
                nc.tensor.matmul(sc[:, 512 + s:1024], kT[p][64:128, ks],
                                 qT[p][64:128, qsv], start=True, stop=True)
                ex = expp.tile([128, 1024], BF16, tag="exp", name="ex")
                if s:
                    exv = ex.rearrange("a (h f) -> a h f", h=2)[:, :, s:512]
                    scv = sc.rearrange("a (h f) -> a h f", h=2)[:, :, s:512]
                else:
                    exv, scv = ex, sc
                if t in SCH_TILES:
                    # Schraudolph fast exp on DVE: one tensor_scalar writing
                    # bf16 bits directly through an int16 bitcast view
                    nc.vector.tensor_scalar(
                        out=exv.bitcast(mybir.dt.int16), in0=scv,
                        scalar1=SCH_A, scalar2=amb[:, i:i + 1],
                        op0=MULT, op1=ADD)
                else:
                    nc.scalar.activation(out=exv, in_=scv, func=EXP,
                                         bias=mask_sb[:, i:i + 1], scale=1.0)
                if crossing:
                    # zero the above-diagonal triangle in the 128-col band
                    ex3 = ex.rearrange("a (h f) -> a h f", h=2)[:, :, s:s + 128]
                    nc.gpsimd.affine_select(
                        out=ex3, in_=ex3, compare_op=GE, fill=0.0,
                        base=0, pattern=[[0, 2], [1, 128]],
                        channel_multiplier=-1,
                    )
                exs[t] = ex

            def emit_av(t):
                p, j, i, ntk = tasks[t]
                crossing = i >= 4 * j
                s = 128 * i - 512 * j if crossing else 0
                if i == 0:
                    out_A = ps_out.tile([128, 512], F32, tag="out", name="out_A")
                    out_B = ps_out.tile([128, 512], F32, tag="out", name="out_B")
                    bstate[(p, j)] = (out_A, out_B)
                out_A, out_B = bstate[(p, j)]
                ex = exs.pop(t)
                nc.tensor.matmul(out_A[:, s:512], vaug[p][i][:, 0:128],
                                 ex[:, s:512],
                                 start=(i == 0), stop=(i == ntk - 1))
                nc.tensor.matmul(out_B[:, s:512], vaug[p][i][:, 128:256],
                                 ex[:, 512 + s:1024],
                                 start=(i == 0), stop=(i == ntk - 1))
                if i == ntk - 1:
                    for h_loc, out_ps in ((0, out_A), (1, out_B)):
                        osb = work.tile([65, 512], F32, tag="u", name="osb")
                        nc.vector.tensor_copy(osb, out_ps[0:65, :])
                        r0 = 130 * p + 65 * h_loc
                        nc.sync.dma_start(
                            out=out[r0:r0 + 65, 512 * j:512 * (j + 1)], in_=osb)

            # prologue: minimal producers for attention(p0, j=3, i=0..)
            emit_qk_chain("q", 0, 3)
            emit_qk_chain("k", 0, 0)
            for tt in range(4):
                emit_v_chain(tt)
            chores_q.append(lambda: emit_qk_chain("k", 0, 1))
            chores_q.append(lambda: emit_v_chain(4))
            chores_q.append(lambda: emit_v_chain(5))
            chores_q.append(lambda: emit_v_chain(6))
            chores_q.append(lambda: emit_qk_chain("k", 0, 2))
            chores_q.append(lambda: emit_v_chain(7))
            chores_q.append(lambda: emit_v_chain(8))
            chores_q.append(lambda: emit_v_chain(9))
            chores_q.append(lambda: emit_qk_chain("k", 0, 3))
            for _tt in range(10, 16):
                chores_q.append(lambda _tt=_tt: emit_v_chain(_tt))
            chores_q.append(lambda: emit_qk_chain("q", 0, 2))
            chores_q.append(lambda: emit_qk_chain("q", 0, 1))
            chores_q.append(lambda: emit_qk_chain("q", 0, 0))
            for t4 in (3, 2, 1, 0):
                chores_q.append(lambda t4=t4: emit_qk_chain("q", 1, t4))
                chores_q.append(lambda t4=t4: emit_qk_chain("k", 1, 3 - t4))

            ntasks = len(tasks)
            for t in range(ntasks + LAG):
                if t < ntasks:
                    emit_scores_exp(t)
                if t - LAG >= 0:
                    emit_av(t - LAG)
                if chores_q:
                    chores_q.pop(0)()

    nc.compile()
    return nc


def _get_nc():
    global _cached_nc
    if _cached_nc is None:
        _cached_nc = _build()
    return _cached_nc


def make_in_maps(hidden_states, attention_mask, Wq, bq, Wk, bk, Wv, bv):
    hidden_states = np.asarray(hidden_states, dtype=np.float32)
    attention_mask = np.asarray(attention_mask, dtype=np.float32)
    Wq = np.asarray(Wq, dtype=np.float32)
    Wk = np.asarray(Wk, dtype=np.float32)
    Wv = np.asarray(Wv, dtype=np.float32)
    bq = np.asarray(bq, dtype=np.float32)
    bk = np.asarray(bk, dtype=np.float32)
    bv = np.asarray(bv, dtype=np.float32)

    bf = ml_dtypes.bfloat16
    in_maps = []
    for c in range(NCORES):
        b, g = divmod(c, 4)
        cs = slice(OC * g, OC * (g + 1))
        hTT = np.ascontiguousarray(hidden_states[b].T).astype(bf)  # [E, S]
        hp = np.empty((128, 32 * 512), dtype=bf)
        for gi, t4 in enumerate((3, 0, 1, 2)):
            for e in range(8):
                hp[:, gi * 4096 + e * 512:gi * 4096 + (e + 1) * 512] = \
                    hTT[e * 128:(e + 1) * 128, t4 * 512:(t4 + 1) * 512]

        def packw(W):
            # e-major: [e-chunk e][all 256 channels] at cols 256e (v path)
            wT = np.ascontiguousarray(W[cs, :].T).astype(bf)  # [E, 256]
            wp = np.empty((128, 2048), dtype=bf)
            for e in range(8):
                wp[:, e * OC:(e + 1) * OC] = wT[e * 128:(e + 1) * 128, :]
            return wp

        def packw_pair(W):
            # pair-major: pair p at cols 1024p, e-chunk e at 128e within
            wT = np.ascontiguousarray(W[cs, :].T).astype(bf)  # [E, 256]
            wp = np.empty((128, 2048), dtype=bf)
            for p in range(2):
                for e in range(8):
                    wp[:, 1024 * p + 128 * e:1024 * p + 128 * (e + 1)] = \
                        wT[e * 128:(e + 1) * 128, 128 * p:128 * (p + 1)]
            return wp

        in_maps.append({
            "hT": hp,
            "wqT": packw_pair(Wq),
            "wkT": packw_pair(Wk),
            "wvT": packw(Wv),
            "bqp": np.ascontiguousarray(bq[cs].reshape(2, 128).T),
            "bkp": np.ascontiguousarray(bk[cs].reshape(2, 128).T),
            "bvf": np.ascontiguousarray(bv[cs]),
            "mask_t": np.ascontiguousarray(
                (attention_mask[b, 0, 0, :] - SHIFT).reshape(NT, 128).T),
        })
    return in_maps


def kernel(hidden_states, attention_mask, Wq, bq, Wk, bk, Wv, bv):
    in_maps = make_in_maps(hidden_states, attention_mask,
                           Wq, bq, Wk, bk, Wv, bv)
    nc = _get_nc()
    res = run_bass_kernel_spmd(nc, in_maps, list(range(NCORES)))

    full = np.empty((B, S, H * D), dtype=np.float32)
    for c in range(NCORES):
        b, g = divmod(c, 4)
        r = res.results[c]["out"]  # [260, S] unnormalized out^T + denom rows
        for p in range(NPAIR):
            for h in range(2):
                blk = r[130 * p + 65 * h: 130 * p + 65 * h + 65]
                c0 = OC * g + 128 * p + 64 * h
                full[b, :, c0:c0 + 64] = (blk[0:64] / blk[64:65]).T
    return full


# revision 21
# speedup vs baseline: 1.0513x; 1.0367x over previous
"""Causal self-attention (B=2, S=2048, E=1024, H=16, D=64) on 8 trn2 NeuronCores.

Sharding: core c = (batch b = c // 4, head-group g = c % 4).  Each core computes
4 heads (one quarter of the 16) for one batch: projections q/k/v for its 256
output channels, then causal flash-style attention over head pairs.

Per-core kernel design (Bass/Tile):
  - Host pre-transposes hidden -> hT [E, S] (bf16) and weight slices -> wT [E, 256]
    (bf16) so all matmul contractions have K on partitions.
  - q/k projections (bf16, PSUM-accumulated over 8 E-chunks) produce qT/kT in
    [d, t] layout (bf16) with scale 1/8 (q) and bias add.
  - v projection produces v in [t, d] layout; DVE copy splits heads into
    v_aug tiles [tk=128, 65*2] with a ones column per head (sum-of-exp trick).
  - scores^T tiles [tk=128, tq<=512] per head via row-tiled bf16 matmuls (the
    two heads of a pair run concurrently on PE row halves, K=64 each).
  - exp via ScalarE activation (mask-2.0 bias per tk partition), bf16 out.
  - causal masking: gpsimd affine_select zeroes the above-diagonal triangle of
    the 128-col diagonal band of crossing tiles after exp.
  - attn @ v_aug accumulates unnormalized out^T [65, tq] in PSUM (bf16 matmuls);
    row 64 is the softmax denominator.  All (block, tile) tasks run through one
    flat software pipeline: scores/exp of tile t issue ahead of the attn@v of
    tile t-LAG so the in-order PE queue never stalls on exp latency.
  - DVE evacuates [65, 512] f32 tiles; DMA to DRAM; the host divides by the
    denominator row and transposes into [t, h*d].
"""

import numpy as np
import ml_dtypes

import concourse.bass as bass
import concourse.mybir as mybir
import concourse.tile as tile
from concourse import bacc
from concourse.bass_utils import run_bass_kernel_spmd

F32 = mybir.dt.float32
BF16 = mybir.dt.bfloat16

B, S, E = 2, 2048, 1024
H, D = 16, 64
NCORES = 8
OC = 256          # output channels per core (4 heads)
NPAIR = 2         # head pairs per core
NT = S // 128     # 16 tk tiles
NT4 = S // 512    # 4 tq blocks
SHIFT = 2.0       # subtracted from scores pre-exp (via mask bias); cancels in
                  # normalization
LAG = 2           # attn@v trails scores/exp by this many pipeline slots
N_WARM = 56       # dummy matmuls during the DMA wait keep the PE HAM clock
                  # gate warm so real chains start at 2.4 GHz
# Schraudolph fast-exp, bf16-direct variant: bf16 bits are the top 16 of f32,
# so int16((A*x + B) / 65536) IS the bf16 bit pattern of exp(x) -- one DVE
# tensor_scalar per tile, output written through a bf16<->int16 bitcast view.
SCH_A = 12102203.16 / 65536.0  # (2^23 / ln 2) >> 16
SCH_B = float(127 * (1 << 23) - 486411) / 65536.0
# chore-free back half: alternate exp between ScalarE and DVE so the two
# engines form parallel exp pipes (ACT alone is the serial wall there)
SCH_TILES = frozenset(t for t in range(29, 80) if t % 2 == 1)

_cached_nc = None


def _patch_ldw_opt():
    # walrus is invoked with --enable-ldw-opt=false hardcoded; LDWEIGHTS
    # scheduling opt measurably tightens back-to-back matmul spacing.
    import os
    if os.environ.get("LDW_OPT", "0") != "1":
        return
    import concourse.bass_utils as _bu
    if getattr(_bu, "_ldw_patched", False):
        return
    _orig = _bu.run_command

    def _patched(argv, **kw):
        argv = ["--enable-ldw-opt=true" if a == "--enable-ldw-opt=false" else a
                for a in argv]
        return _orig(argv, **kw)

    _bu.run_command = _patched
    _bu._ldw_patched = True


def _build():
    _patch_ldw_opt()
    nc = bacc.Bacc()

    hT = nc.declare_dram_parameter("hT", [128, 32 * 512], BF16, isOutput=False)
    wqT = nc.declare_dram_parameter("wqT", [128, 2048], BF16, isOutput=False)
    wkT = nc.declare_dram_parameter("wkT", [128, 2048], BF16, isOutput=False)
    wvT = nc.declare_dram_parameter("wvT", [128, 2048], BF16, isOutput=False)
    bqp = nc.declare_dram_parameter("bqp", [128, 2], F32, isOutput=False)
    bkp = nc.declare_dram_parameter("bkp", [128, 2], F32, isOutput=False)
    bvf = nc.declare_dram_parameter("bvf", [OC], F32, isOutput=False)
    mask_t = nc.declare_dram_parameter("mask_t", [128, NT], F32, isOutput=False)
    out = nc.declare_dram_parameter("out", [4 * 65, S], F32, isOutput=True)

    EXP = mybir.ActivationFunctionType.Exp
    ADD = mybir.AluOpType.add
    MULT = mybir.AluOpType.mult
    GE = mybir.AluOpType.is_ge
    I32 = mybir.dt.int32

    with tile.TileContext(nc) as tc:
        with (
            tc.tile_pool(name="cst", bufs=1) as cst,
            tc.tile_pool(name="work", bufs=4) as work,
            tc.tile_pool(name="expp", bufs=6) as expp,
            # one shared 3-deep [128,1024] psum pool for BOTH projection
            # chains and score tiles: chains dominate the chore-dense front
            # half, scores need the 3-tile runahead in the chore-free back
            # half.  3x2 banks + 2 out banks = all 8 PSUM banks.
            tc.tile_pool(name="ps_sc", bufs=3, space="PSUM") as ps_sc,
            tc.tile_pool(name="ps_out", bufs=2, space="PSUM") as ps_out,
        ):
            # ---- PE warmup: dummy matmuls during the DMA wait (no DMA deps)
            # keep the HAM activity window busy so real chains start warm ----
            dum = cst.tile([128, 64], BF16, tag="dum")
            nc.vector.memset(dum, 0.5)
            for _w in range(N_WARM):
                ps_d = ps_sc.tile([128, 1024], F32, tag="sc", name="warm")
                nc.tensor.matmul(ps_d[0:64, 0:64], dum, dum, start=True, stop=True)

            # ---- big resident inputs, host-packed in consumption order:
            # wq/wk pair-major (pair p at cols 1024p) so the first chains need
            # only the pair-0 piece; hT in t4 groups 3,0,1,2. ----
            G = {3: 0, 0: 1, 1: 2, 2: 3}  # t4 -> group position
            hT_big = cst.tile([128, 32 * 512], BF16, tag="hT_big")
            wq_big = cst.tile([128, 2048], BF16, tag="wq_big")
            wk_big = cst.tile([128, 2048], BF16, tag="wk_big")
            wv_big = cst.tile([128, 2048], BF16, tag="wv_big")
            # first critical pieces go out on idle engine queues — the sync
            # queue is busy with semaphore setup for several us at kernel start
            nc.scalar.dma_start(out=wq_big[:, 0:1024], in_=wqT[:, 0:1024])
            nc.scalar.dma_start(out=hT_big[:, 0:2048], in_=hT[:, 0:2048])
            nc.gpsimd.dma_start(out=hT_big[:, 2048:4096], in_=hT[:, 2048:4096])
            nc.scalar.dma_start(out=wk_big[:, 0:1024], in_=wkT[:, 0:1024])
            bq_sb = cst.tile([128, 2], F32, tag="bq")
            nc.gpsimd.dma_start(out=bq_sb, in_=bqp[:, :])
            bk_sb = cst.tile([128, 2], F32, tag="bk")
            nc.gpsimd.dma_start(out=bk_sb, in_=bkp[:, :])
            mask_sb = cst.tile([128, NT], F32, tag="mask")
            nc.gpsimd.dma_start(out=mask_sb, in_=mask_t[:, :])
            nc.sync.dma_start(out=hT_big[:, 4096:6144], in_=hT[:, 4096:6144])
            nc.sync.dma_start(out=hT_big[:, 6144:8192], in_=hT[:, 6144:8192])
            nc.sync.dma_start(out=wv_big[:, 0:1024], in_=wvT[:, 0:1024])
            nc.sync.dma_start(out=wv_big[:, 1024:2048], in_=wvT[:, 1024:2048])
            bv_sb = cst.tile([128, OC], F32, tag="bv")
            nc.gpsimd.dma_start(out=bv_sb, in_=bvf[:].partition_broadcast(128))
            nc.sync.dma_start(out=hT_big[:, 8192:12288], in_=hT[:, 8192:12288])
            nc.sync.dma_start(out=wq_big[:, 1024:2048], in_=wqT[:, 1024:2048])
            nc.sync.dma_start(out=wk_big[:, 1024:2048], in_=wkT[:, 1024:2048])
            nc.sync.dma_start(out=hT_big[:, 12288:16384], in_=hT[:, 12288:16384])

            # Schraudolph per-partition affine term: A*mask' + B
            amb = cst.tile([128, NT], F32, tag="amb")
            nc.vector.tensor_scalar(out=amb, in0=mask_sb, scalar1=SCH_A,
                                    scalar2=SCH_B, op0=MULT, op1=ADD)

            hT32 = [[hT_big[:, G[t4] * 4096 + e * 512: G[t4] * 4096 + (e + 1) * 512]
                     for t4 in range(NT4)] for e in range(8)]
            # wq/wk pair-major: [pair p][e-chunk] at cols 1024p + 128e
            wqk_sb = {nm: [[big[:, 1024 * p + 128 * e: 1024 * p + 128 * (e + 1)]
                            for e in range(8)] for p in range(NPAIR)]
                      for nm, big in (("q", wq_big), ("k", wk_big))}
            wv_sb = [wv_big[:, e * OC:(e + 1) * OC] for e in range(8)]

            # ---- persistent intermediates ----
            qT = [cst.tile([128, S], BF16, tag=f"qT{p}", name=f"qT{p}") for p in range(NPAIR)]
            kT = [cst.tile([128, S], BF16, tag=f"kT{p}", name=f"kT{p}") for p in range(NPAIR)]
            vaug = [[cst.tile([128, 256], BF16, tag=f"va{p}_{tt}", name=f"va{p}_{tt}")
                     for tt in range(NT)] for p in range(NPAIR)]

            def emit_qk_chain(nm, p, t4):
                dst = qT[p] if nm == "q" else kT[p]
                b_sb = bq_sb if nm == "q" else bk_sb
                ts = slice(512 * t4, 512 * (t4 + 1))
                ps_qk = ps_sc.tile([128, 1024], F32, tag="sc", name="ps_qk")[:, 0:512]
                for e in range(8):
                    nc.tensor.matmul(
                        ps_qk,
                        wqk_sb[nm][p][e],
                        hT32[e][t4],
                        start=(e == 0), stop=(e == 7),
                    )
                if nm == "q":
                    nc.vector.tensor_scalar(
                        out=dst[:, ts], in0=ps_qk,
                        scalar1=0.125, scalar2=b_sb[:, p:p + 1],
                        op0=MULT, op1=ADD,
                    )
                else:
                    nc.vector.tensor_scalar_add(
                        out=dst[:, ts], in0=ps_qk, scalar1=b_sb[:, p:p + 1],
                    )

            def emit_v_chain(tt):
                t4v, r4 = divmod(tt, 4)
                rs = slice(128 * r4, 128 * (r4 + 1))
                ps_v = ps_sc.tile([128, 1024], F32, tag="sc", name="ps_v")[:, 0:OC]
                for e in range(8):
                    nc.tensor.matmul(
                        ps_v,
                        hT32[e][t4v][:, rs],
                        wv_sb[e],
                        start=(e == 0), stop=(e == 7),
                    )
                for p in range(NPAIR):
                    po = 128 * p
                    vt = vaug[p][tt]
                    vt3 = vt.rearrange("a (h c) -> a h c", h=2)[:, :, 0:64]
                    ps3 = ps_v[:, po:po + 128].rearrange("a (h c) -> a h c", h=2)
                    bv3 = bv_sb[:, po:po + 128].rearrange("a (h c) -> a h c", h=2)
                    nc.vector.tensor_add(vt3, ps3, bv3)
                    # ones column for the sum-of-exp denominator; cols 65:128
                    # stay uninitialized (their psum rows are never read)
                    nc.vector.memset(
                        vt.rearrange("a (h c) -> a h c", h=2)[:, :, 64:65], 1.0)

            chores_q = []

            # ---- flat attention pipeline across all (p, j) blocks ----
            blocks = [(0, 3), (0, 2), (1, 3), (0, 1), (1, 2), (0, 0), (1, 1), (1, 0)]
            tasks = []  # (p, j, i, ntk)
            for p, j in blocks:
                ntk = 4 * (j + 1)
                for i in range(ntk):
                    tasks.append((p, j, i, ntk))
            bstate = {}  # (p, j) -> (out_A, out_B)
            exs = {}     # flat index -> ex tile

            def emit_scores_exp(t):
                p, j, i, ntk = tasks[t]
                crossing = i >= 4 * j
                s = 128 * i - 512 * j if crossing else 0
                ks = slice(128 * i, 128 * (i + 1))
                qsv = slice(512 * j + s, 512 * (j + 1))
                sc = ps_sc.tile([128, 1024], F32, tag="sc", name="sc")
                nc.tensor.matmul(sc[:, s:512], kT[p][0:64, ks],
                                 qT[p][0:64, qsv], start=True, stop=True)
                nc.tensor.matmul(sc[:, 512 + s:1024], kT[p][64:128, ks],
                                 qT[p][64:128, qsv], start=True, stop=True)
                ex = expp.tile([128, 1024], BF16, tag="exp", name="ex")
                if s:
                    exv = ex.rearrange("a (h f) -> a h f", h=2)[:, :, s:512]
                    scv = sc.rearrange("a (h f) -> a h f", h=2)[:, :, s:512]
                else:
                    exv, scv = ex, sc
                if t in SCH_TILES:
                    # Schraudolph fast exp on DVE: one tensor_scalar writing
                    # bf16 bits directly through an int16 bitcast view
                    nc.vector.tensor_scalar(
                        out=exv.bitcast(mybir.dt.int16), in0=scv,
                        scalar1=SCH_A, scalar2=amb[:, i:i + 1],
                        op0=MULT, op1=ADD)
                else:
                    nc.scalar.activation(out=exv, in_=scv, func=EXP,
                                         bias=mask_sb[:, i:i + 1], scale=1.0)
                if crossing:
                    # zero the above-diagonal triangle in the 128-col band
                    ex3 = ex.rearrange("a (h f) -> a h f", h=2)[:, :, s:s + 128]
                    nc.gpsimd.affine_select(
                        out=ex3, in_=ex3, compare_op=GE, fill=0.0,
                        base=0, pattern=[[0, 2], [1, 128]],
                        channel_multiplier=-1,
                    )
                exs[t] = ex

            def emit_av(t):
                p, j, i, ntk = tasks[t]
                crossing = i >= 4 * j
                s = 128 * i - 512 * j if crossing else 0
                if i == 0:
                    out_A = ps_out.tile([128, 512], F32, tag="out", name="out_A")
                    out_B = ps_out.tile([128, 512], F32, tag="out", name="out_B")
                    bstate[(p, j)] = (out_A, out_B)
                out_A, out_B = bstate[(p, j)]
                ex = exs.pop(t)
                nc.tensor.matmul(out_A[:, s:512], vaug[p][i][:, 0:128],
                                 ex[:, s:512],
                                 start=(i == 0), stop=(i == ntk - 1))
                nc.tensor.matmul(out_B[:, s:512], vaug[p][i][:, 128:256],
                                 ex[:, 512 + s:1024],
                                 start=(i == 0), stop=(i == ntk - 1))
                if i == ntk - 1:
                    for h_loc, out_ps in ((0, out_A), (1, out_B)):
                        osb = work.tile([65, 512], F32, tag="u", name="osb")
                        nc.vector.tensor_copy(osb, out_ps[0:65, :])
                        r0 = 130 * p + 65 * h_loc
                        nc.sync.dma_start(
                            out=out[r0:r0 + 65, 512 * j:512 * (j + 1)], in_=osb)

            # prologue: minimal producers for attention(p0, j=3, i=0..)
            emit_qk_chain("q", 0, 3)
            emit_qk_chain("k", 0, 0)
            for tt in range(4):
                emit_v_chain(tt)
            chores_q.append(lambda: emit_qk_chain("k", 0, 1))
            chores_q.append(lambda: emit_v_chain(4))
            chores_q.append(lambda: emit_v_chain(5))
            chores_q.append(lambda: emit_v_chain(6))
            chores_q.append(lambda: emit_qk_chain("k", 0, 2))
            chores_q.append(lambda: emit_v_chain(7))
            chores_q.append(lambda: emit_v_chain(8))
            chores_q.append(lambda: emit_v_chain(9))
            chores_q.append(lambda: emit_qk_chain("k", 0, 3))
            for _tt in range(10, 16):
                chores_q.append(lambda _tt=_tt: emit_v_chain(_tt))
            chores_q.append(lambda: emit_qk_chain("q", 0, 2))
            chores_q.append(lambda: emit_qk_chain("q", 0, 1))
            chores_q.append(lambda: emit_qk_chain("q", 0, 0))
            for t4 in (3, 2, 1, 0):
                chores_q.append(lambda t4=t4: emit_qk_chain("q", 1, t4))
                chores_q.append(lambda t4=t4: emit_qk_chain("k", 1, 3 - t4))

            ntasks = len(tasks)
            for t in range(ntasks + LAG):
                if t < ntasks:
                    emit_scores_exp(t)
                if t - LAG >= 0:
                    emit_av(t - LAG)
                if chores_q:
                    chores_q.pop(0)()

    nc.compile()
    return nc


def _get_nc():
    global _cached_nc
    if _cached_nc is None:
        _cached_nc = _build()
    return _cached_nc


def make_in_maps(hidden_states, attention_mask, Wq, bq, Wk, bk, Wv, bv):
    hidden_states = np.asarray(hidden_states, dtype=np.float32)
    attention_mask = np.asarray(attention_mask, dtype=np.float32)
    Wq = np.asarray(Wq, dtype=np.float32)
    Wk = np.asarray(Wk, dtype=np.float32)
    Wv = np.asarray(Wv, dtype=np.float32)
    bq = np.asarray(bq, dtype=np.float32)
    bk = np.asarray(bk, dtype=np.float32)
    bv = np.asarray(bv, dtype=np.float32)

    bf = ml_dtypes.bfloat16
    in_maps = []
    for c in range(NCORES):
        b, g = divmod(c, 4)
        cs = slice(OC * g, OC * (g + 1))
        hTT = np.ascontiguousarray(hidden_states[b].T).astype(bf)  # [E, S]
        hp = np.empty((128, 32 * 512), dtype=bf)
        for gi, t4 in enumerate((3, 0, 1, 2)):
            for e in range(8):
                hp[:, gi * 4096 + e * 512:gi * 4096 + (e + 1) * 512] = \
                    hTT[e * 128:(e + 1) * 128, t4 * 512:(t4 + 1) * 512]

        def packw(W):
            # e-major: [e-chunk e][all 256 channels] at cols 256e (v path)
            wT = np.ascontiguousarray(W[cs, :].T).astype(bf)  # [E, 256]
            wp = np.empty((128, 2048), dtype=bf)
            for e in range(8):
                wp[:, e * OC:(e + 1) * OC] = wT[e * 128:(e + 1) * 128, :]
            return wp

        def packw_pair(W):
            # pair-major: pair p at cols 1024p, e-chunk e at 128e within
            wT = np.ascontiguousarray(W[cs, :].T).astype(bf)  # [E, 256]
            wp = np.empty((128, 2048), dtype=bf)
            for p in range(2):
                for e in range(8):
                    wp[:, 1024 * p + 128 * e:1024 * p + 128 * (e + 1)] = \
                        wT[e * 128:(e + 1) * 128, 128 * p:128 * (p + 1)]
            return wp

        in_maps.append({
            "hT": hp,
            "wqT": packw_pair(Wq),
            "wkT": packw_pair(Wk),
            "wvT": packw(Wv),
            "bqp": np.ascontiguousarray(bq[cs].reshape(2, 128).T),
            "bkp": np.ascontiguousarray(bk[cs].reshape(2, 128).T),
            "bvf": np.ascontiguousarray(bv[cs]),
            "mask_t": np.ascontiguousarray(
                (attention_mask[b, 0, 0, :] - SHIFT).reshape(NT, 128).T),
        })
    return in_maps


def kernel(hidden_states, attention_mask, Wq, bq, Wk, bk, Wv, bv):
    in_maps = make_in_maps(hidden_states, attention_mask,
                           Wq, bq, Wk, bk, Wv, bv)
    nc = _get_nc()
    res = run_bass_kernel_spmd(nc, in_maps, list(range(NCORES)))

    full = np.empty((B, S, H * D), dtype=np.float32)
    for c in range(NCORES):
        b, g = divmod(c, 4)
        r = res.results[c]["out"]  # [260, S] unnormalized out^T + denom rows
        for p in range(NPAIR):
            for h in range(2):
                blk = r[130 * p + 65 * h: 130 * p + 65 * h + 65]
                c0 = OC * g + 128 * p + 64 * h
                full[b, :, c0:c0 + 64] = (blk[0:64] / blk[64:65]).T
    return full


# revision 24
# speedup vs baseline: 1.0609x; 1.0091x over previous
"""Causal self-attention (B=2, S=2048, E=1024, H=16, D=64) on 8 trn2 NeuronCores.

Sharding: core c = (batch b = c // 4, head-group g = c % 4).  Each core computes
4 heads (one quarter of the 16) for one batch: projections q/k/v for its 256
output channels, then causal flash-style attention over head pairs.

Per-core kernel design (Bass/Tile):
  - Host pre-transposes hidden -> hT [E, S] (bf16) and weight slices -> wT [E, 256]
    (bf16) so all matmul contractions have K on partitions.
  - q/k projections (bf16, PSUM-accumulated over 8 E-chunks) produce qT/kT in
    [d, t] layout (bf16) with scale 1/8 (q) and bias add.
  - v projection produces v in [t, d] layout; DVE copy splits heads into
    v_aug tiles [tk=128, 65*2] with a ones column per head (sum-of-exp trick).
  - scores^T tiles [tk=128, tq<=512] per head via row-tiled bf16 matmuls (the
    two heads of a pair run concurrently on PE row halves, K=64 each).
  - exp via ScalarE activation (mask-2.0 bias per tk partition), bf16 out.
  - causal masking: gpsimd affine_select zeroes the above-diagonal triangle of
    the 128-col diagonal band of crossing tiles after exp.
  - attn @ v_aug accumulates unnormalized out^T [65, tq] in PSUM (bf16 matmuls);
    row 64 is the softmax denominator.  All (block, tile) tasks run through one
    flat software pipeline: scores/exp of tile t issue ahead of the attn@v of
    tile t-LAG so the in-order PE queue never stalls on exp latency.
  - DVE evacuates [65, 512] f32 tiles; DMA to DRAM; the host divides by the
    denominator row and transposes into [t, h*d].
"""

import numpy as np
import ml_dtypes

import concourse.bass as bass
import concourse.mybir as mybir
import concourse.tile as tile
from concourse import bacc
from concourse.bass_utils import run_bass_kernel_spmd

F32 = mybir.dt.float32
BF16 = mybir.dt.bfloat16

B, S, E = 2, 2048, 1024
H, D = 16, 64
NCORES = 8
OC = 256          # output channels per core (4 heads)
NPAIR = 2         # head pairs per core
NT = S // 128     # 16 tk tiles
NT4 = S // 512    # 4 tq blocks
SHIFT = 2.0       # subtracted from scores pre-exp (via mask bias); cancels in
                  # normalization
LAG = 2           # attn@v trails scores/exp by this many pipeline slots
N_WARM = 20       # dummy matmuls during the DMA wait keep the PE HAM clock
                  # gate warm so real chains start at 2.4 GHz
# Schraudolph fast-exp, bf16-direct variant: bf16 bits are the top 16 of f32,
# so int16((A*x + B) / 65536) IS the bf16 bit pattern of exp(x) -- one DVE
# tensor_scalar per tile, output written through a bf16<->int16 bitcast view.
SCH_A = 12102203.16 / 65536.0  # (2^23 / ln 2) >> 16
SCH_B = float(127 * (1 << 23) - 486411) / 65536.0
# chore-free back half: alternate exp between ScalarE and DVE so the two
# engines form parallel exp pipes (ACT alone is the serial wall there)
SCH_TILES = frozenset(t for t in range(29, 80) if t % 2 == 1)

_cached_nc = None


def _patch_ldw_opt():
    # walrus is invoked with --enable-ldw-opt=false hardcoded; LDWEIGHTS
    # scheduling opt measurably tightens back-to-back matmul spacing.
    import os
    if os.environ.get("LDW_OPT", "0") != "1":
        return
    import concourse.bass_utils as _bu
    if getattr(_bu, "_ldw_patched", False):
        return
    _orig = _bu.run_command

    def _patched(argv, **kw):
        argv = ["--enable-ldw-opt=true" if a == "--enable-ldw-opt=false" else a
                for a in argv]
        return _orig(argv, **kw)

    _bu.run_command = _patched
    _bu._ldw_patched = True


def _build():
    _patch_ldw_opt()
    nc = bacc.Bacc()

    hT = nc.declare_dram_parameter("hT", [128, 32 * 512], BF16, isOutput=False)
    wqT = nc.declare_dram_parameter("wqT", [128, 2048], BF16, isOutput=False)
    wkT = nc.declare_dram_parameter("wkT", [128, 2048], BF16, isOutput=False)
    wvT = nc.declare_dram_parameter("wvT", [128, 2048], BF16, isOutput=False)
    bqp = nc.declare_dram_parameter("bqp", [128, 2], F32, isOutput=False)
    bkp = nc.declare_dram_parameter("bkp", [128, 2], F32, isOutput=False)
    bvf = nc.declare_dram_parameter("bvf", [OC], F32, isOutput=False)
    mask_t = nc.declare_dram_parameter("mask_t", [128, NT], F32, isOutput=False)
    out = nc.declare_dram_parameter("out", [4 * 65, S], F32, isOutput=True)

    EXP = mybir.ActivationFunctionType.Exp
    ADD = mybir.AluOpType.add
    MULT = mybir.AluOpType.mult
    GE = mybir.AluOpType.is_ge
    I32 = mybir.dt.int32

    with tile.TileContext(nc) as tc:
        with (
            tc.tile_pool(name="cst", bufs=1) as cst,
            tc.tile_pool(name="work", bufs=4) as work,
            tc.tile_pool(name="expp", bufs=6) as expp,
            # one shared 3-deep [128,1024] psum pool for BOTH projection
            # chains and score tiles: chains dominate the chore-dense front
            # half, scores need the 3-tile runahead in the chore-free back
            # half.  3x2 banks + 2 out banks = all 8 PSUM banks.
            tc.tile_pool(name="ps_sc", bufs=3, space="PSUM") as ps_sc,
            tc.tile_pool(name="ps_out", bufs=2, space="PSUM") as ps_out,
        ):
            # ---- PE warmup: dummy matmuls during the DMA wait (no DMA deps)
            # keep the HAM activity window busy so real chains start warm ----
            dum = cst.tile([128, 64], BF16, tag="dum")
            nc.vector.memset(dum, 0.5)
            for _w in range(N_WARM):
                ps_d = ps_sc.tile([128, 1024], F32, tag="sc", name="warm")
                nc.tensor.matmul(ps_d[0:64, 0:64], dum, dum, start=True, stop=True)

            # ---- big resident inputs, host-packed in consumption order:
            # wq/wk pair-major (pair p at cols 1024p) so the first chains need
            # only the pair-0 piece; hT in t4 groups 3,0,1,2. ----
            G = {3: 0, 0: 1, 1: 2, 2: 3}  # t4 -> group position
            hT_big = cst.tile([128, 32 * 512], BF16, tag="hT_big")
            wq_big = cst.tile([128, 2048], BF16, tag="wq_big")
            wk_big = cst.tile([128, 2048], BF16, tag="wk_big")
            wv_big = cst.tile([128, 2048], BF16, tag="wv_big")
            # all early-critical pieces issue from the scalar queue in exact
            # consumption order — the sync queue is busy with semaphore setup
            # for ~10us at kernel start; small consts go via gpsimd
            nc.scalar.dma_start(out=wq_big[:, 0:1024], in_=wqT[:, 0:1024])
            nc.scalar.dma_start(out=hT_big[:, 0:2048], in_=hT[:, 0:2048])
            nc.scalar.dma_start(out=hT_big[:, 2048:4096], in_=hT[:, 2048:4096])
            nc.scalar.dma_start(out=wk_big[:, 0:1024], in_=wkT[:, 0:1024])
            nc.scalar.dma_start(out=hT_big[:, 4096:6144], in_=hT[:, 4096:6144])
            nc.scalar.dma_start(out=hT_big[:, 6144:8192], in_=hT[:, 6144:8192])
            nc.scalar.dma_start(out=wv_big[:, 0:1024], in_=wvT[:, 0:1024])
            nc.scalar.dma_start(out=wv_big[:, 1024:2048], in_=wvT[:, 1024:2048])
            nc.scalar.dma_start(out=hT_big[:, 8192:10240], in_=hT[:, 8192:10240])
            nc.scalar.dma_start(out=hT_big[:, 10240:12288], in_=hT[:, 10240:12288])
            bq_sb = cst.tile([128, 2], F32, tag="bq")
            nc.gpsimd.dma_start(out=bq_sb, in_=bqp[:, :])
            bk_sb = cst.tile([128, 2], F32, tag="bk")
            nc.gpsimd.dma_start(out=bk_sb, in_=bkp[:, :])
            mask_sb = cst.tile([128, NT], F32, tag="mask")
            nc.gpsimd.dma_start(out=mask_sb, in_=mask_t[:, :])
            bv_sb = cst.tile([128, OC], F32, tag="bv")
            nc.gpsimd.dma_start(out=bv_sb, in_=bvf[:].partition_broadcast(128))
            nc.sync.dma_start(out=hT_big[:, 12288:16384], in_=hT[:, 12288:16384])
            nc.sync.dma_start(out=wq_big[:, 1024:2048], in_=wqT[:, 1024:2048])
            nc.sync.dma_start(out=wk_big[:, 1024:2048], in_=wkT[:, 1024:2048])

            # Schraudolph per-partition affine term: A*mask' + B
            amb = cst.tile([128, NT], F32, tag="amb")
            nc.vector.tensor_scalar(out=amb, in0=mask_sb, scalar1=SCH_A,
                                    scalar2=SCH_B, op0=MULT, op1=ADD)

            hT32 = [[hT_big[:, G[t4] * 4096 + e * 512: G[t4] * 4096 + (e + 1) * 512]
                     for t4 in range(NT4)] for e in range(8)]
            # wq/wk pair-major: [pair p][e-chunk] at cols 1024p + 128e
            wqk_sb = {nm: [[big[:, 1024 * p + 128 * e: 1024 * p + 128 * (e + 1)]
                            for e in range(8)] for p in range(NPAIR)]
                      for nm, big in (("q", wq_big), ("k", wk_big))}
            wv_sb = [wv_big[:, e * OC:(e + 1) * OC] for e in range(8)]

            # ---- persistent intermediates ----
            qT = [cst.tile([128, S], BF16, tag=f"qT{p}", name=f"qT{p}") for p in range(NPAIR)]
            kT = [cst.tile([128, S], BF16, tag=f"kT{p}", name=f"kT{p}") for p in range(NPAIR)]
            vaug = [[cst.tile([128, 256], BF16, tag=f"va{p}_{tt}", name=f"va{p}_{tt}")
                     for tt in range(NT)] for p in range(NPAIR)]

            def emit_qk_chain(nm, p, t4):
                dst = qT[p] if nm == "q" else kT[p]
                b_sb = bq_sb if nm == "q" else bk_sb
                ts = slice(512 * t4, 512 * (t4 + 1))
                ps_qk = ps_sc.tile([128, 1024], F32, tag="sc", name="ps_qk")[:, 0:512]
                for e in range(8):
                    nc.tensor.matmul(
                        ps_qk,
                        wqk_sb[nm][p][e],
                        hT32[e][t4],
                        start=(e == 0), stop=(e == 7),
                    )
                if nm == "q":
                    nc.vector.tensor_scalar(
                        out=dst[:, ts], in0=ps_qk,
                        scalar1=0.125, scalar2=b_sb[:, p:p + 1],
                        op0=MULT, op1=ADD,
                    )
                else:
                    nc.vector.tensor_scalar_add(
                        out=dst[:, ts], in0=ps_qk, scalar1=b_sb[:, p:p + 1],
                    )

            def emit_v_chain(tt):
                t4v, r4 = divmod(tt, 4)
                rs = slice(128 * r4, 128 * (r4 + 1))
                ps_v = ps_sc.tile([128, 1024], F32, tag="sc", name="ps_v")[:, 0:OC]
                for e in range(8):
                    nc.tensor.matmul(
                        ps_v,
                        hT32[e][t4v][:, rs],
                        wv_sb[e],
                        start=(e == 0), stop=(e == 7),
                    )
                for p in range(NPAIR):
                    po = 128 * p
                    vt = vaug[p][tt]
                    vt3 = vt.rearrange("a (h c) -> a h c", h=2)[:, :, 0:64]
                    ps3 = ps_v[:, po:po + 128].rearrange("a (h c) -> a h c", h=2)
                    bv3 = bv_sb[:, po:po + 128].rearrange("a (h c) -> a h c", h=2)
                    nc.vector.tensor_add(vt3, ps3, bv3)
                    # ones column for the sum-of-exp denominator; cols 65:128
                    # stay uninitialized (their psum rows are never read)
                    nc.vector.memset(
                        vt.rearrange("a (h c) -> a h c", h=2)[:, :, 64:65], 1.0)

            chores_q = []

            # ---- flat attention pipeline across all (p, j) blocks ----
            blocks = [(0, 3), (0, 2), (1, 3), (0, 1), (1, 2), (0, 0), (1, 1), (1, 0)]
            tasks = []  # (p, j, i, ntk)
            for p, j in blocks:
                ntk = 4 * (j + 1)
                for i in range(ntk):
                    tasks.append((p, j, i, ntk))
            bstate = {}  # (p, j) -> (out_A, out_B)
            exs = {}     # flat index -> ex tile

            def emit_scores_exp(t):
                p, j, i, ntk = tasks[t]
                crossing = i >= 4 * j
                s = 128 * i - 512 * j if crossing else 0
                ks = slice(128 * i, 128 * (i + 1))
                qsv = slice(512 * j + s, 512 * (j + 1))
                sc = ps_sc.tile([128, 1024], F32, tag="sc", name="sc")
                nc.tensor.matmul(sc[:, s:512], kT[p][0:64, ks],
                                 qT[p][0:64, qsv], start=True, stop=True)
                nc.tensor.matmul(sc[:, 512 + s:1024], kT[p][64:128, ks],
                                 qT[p][64:128, qsv], start=True, stop=True)
                ex = expp.tile([128, 1024], BF16, tag="exp", name="ex")
                if s:
                    exv = ex.rearrange("a (h f) -> a h f", h=2)[:, :, s:512]
                    scv = sc.rearrange("a (h f) -> a h f", h=2)[:, :, s:512]
                else:
                    exv, scv = ex, sc
                if t in SCH_TILES:
                    # Schraudolph fast exp on DVE: one tensor_scalar writing
                    # bf16 bits directly through an int16 bitcast view
                    nc.vector.tensor_scalar(
                        out=exv.bitcast(mybir.dt.int16), in0=scv,
                        scalar1=SCH_A, scalar2=amb[:, i:i + 1],
                        op0=MULT, op1=ADD)
                else:
                    nc.scalar.activation(out=exv, in_=scv, func=EXP,
                                         bias=mask_sb[:, i:i + 1], scale=1.0)
                if crossing:
                    # zero the above-diagonal triangle in the 128-col band
                    ex3 = ex.rearrange("a (h f) -> a h f", h=2)[:, :, s:s + 128]
                    nc.gpsimd.affine_select(
                        out=ex3, in_=ex3, compare_op=GE, fill=0.0,
                        base=0, pattern=[[0, 2], [1, 128]],
                        channel_multiplier=-1,
                    )
                exs[t] = ex

            def emit_av(t):
                p, j, i, ntk = tasks[t]
                crossing = i >= 4 * j
                s = 128 * i - 512 * j if crossing else 0
                if i == 0:
                    out_A = ps_out.tile([128, 512], F32, tag="out", name="out_A")
                    out_B = ps_out.tile([128, 512], F32, tag="out", name="out_B")
                    bstate[(p, j)] = (out_A, out_B)
                out_A, out_B = bstate[(p, j)]
                ex = exs.pop(t)
                nc.tensor.matmul(out_A[:, s:512], vaug[p][i][:, 0:128],
                                 ex[:, s:512],
                                 start=(i == 0), stop=(i == ntk - 1))
                nc.tensor.matmul(out_B[:, s:512], vaug[p][i][:, 128:256],
                                 ex[:, 512 + s:1024],
                                 start=(i == 0), stop=(i == ntk - 1))
                if i == ntk - 1:
                    for h_loc, out_ps in ((0, out_A), (1, out_B)):
                        osb = work.tile([65, 512], F32, tag="u", name="osb")
                        nc.vector.tensor_copy(osb, out_ps[0:65, :])
                        r0 = 130 * p + 65 * h_loc
                        nc.sync.dma_start(
                            out=out[r0:r0 + 65, 512 * j:512 * (j + 1)], in_=osb)

            # prologue: minimal producers for the first scores (p0, j=3)
            emit_qk_chain("q", 0, 3)
            emit_qk_chain("k", 0, 0)
            # chores with emission deadlines (flat slot by which the producer
            # must be emitted so its consumer sees the write):
            #   v(tt) before av(tt) at slot tt+2; k(0,x) before sc(4x);
            #   q(0,j') before its block; pair-1 chains before slot 28
            chores_q.extend([
                (2, lambda: emit_v_chain(0)),
                (3, lambda: emit_qk_chain("k", 0, 1)),
                (3, lambda: emit_v_chain(1)),
                (4, lambda: emit_v_chain(2)),
                (5, lambda: emit_v_chain(3)),
                (6, lambda: emit_v_chain(4)),
                (7, lambda: emit_qk_chain("k", 0, 2)),
                (7, lambda: emit_v_chain(5)),
                (8, lambda: emit_v_chain(6)),
                (9, lambda: emit_v_chain(7)),
                (10, lambda: emit_v_chain(8)),
                (11, lambda: emit_qk_chain("k", 0, 3)),
                (11, lambda: emit_v_chain(9)),
                (12, lambda: emit_v_chain(10)),
                (13, lambda: emit_v_chain(11)),
                (14, lambda: emit_v_chain(12)),
                (15, lambda: emit_qk_chain("q", 0, 2)),
                (15, lambda: emit_v_chain(13)),
                (16, lambda: emit_v_chain(14)),
                (17, lambda: emit_v_chain(15)),
                (20, lambda: emit_qk_chain("q", 1, 3)),
                (21, lambda: emit_qk_chain("k", 1, 0)),
                (22, lambda: emit_qk_chain("q", 1, 2)),
                (23, lambda: emit_qk_chain("k", 1, 1)),
                (25, lambda: emit_qk_chain("k", 1, 2)),
                (27, lambda: emit_qk_chain("k", 1, 3)),
                (30, lambda: emit_qk_chain("q", 1, 1)),
                (35, lambda: emit_qk_chain("q", 1, 0)),
                (43, lambda: emit_qk_chain("q", 0, 1)),
                (63, lambda: emit_qk_chain("q", 0, 0)),
            ])

            ntasks = len(tasks)
            for t in range(ntasks + LAG):
                if t < ntasks:
                    emit_scores_exp(t)
                # chores sit between scores and attn@v: they fill the exp
                # latency window; pop everything due plus one for pacing
                popped = False
                while chores_q and chores_q[0][0] <= t:
                    chores_q.pop(0)[1]()
                    popped = True
                if chores_q and not popped:
                    chores_q.pop(0)[1]()
                if t - LAG >= 0:
                    emit_av(t - LAG)

    nc.compile()
    return nc


def _get_nc():
    global _cached_nc
    if _cached_nc is None:
        _cached_nc = _build()
    return _cached_nc


def make_in_maps(hidden_states, attention_mask, Wq, bq, Wk, bk, Wv, bv):
    hidden_states = np.asarray(hidden_states, dtype=np.float32)
    attention_mask = np.asarray(attention_mask, dtype=np.float32)
    Wq = np.asarray(Wq, dtype=np.float32)
    Wk = np.asarray(Wk, dtype=np.float32)
    Wv = np.asarray(Wv, dtype=np.float32)
    bq = np.asarray(bq, dtype=np.float32)
    bk = np.asarray(bk, dtype=np.float32)
    bv = np.asarray(bv, dtype=np.float32)

    bf = ml_dtypes.bfloat16
    in_maps = []
    for c in range(NCORES):
        b, g = divmod(c, 4)
        cs = slice(OC * g, OC * (g + 1))
        hTT = np.ascontiguousarray(hidden_states[b].T).astype(bf)  # [E, S]
        hp = np.empty((128, 32 * 512), dtype=bf)
        for gi, t4 in enumerate((3, 0, 1, 2)):
            for e in range(8):
                hp[:, gi * 4096 + e * 512:gi * 4096 + (e + 1) * 512] = \
                    hTT[e * 128:(e + 1) * 128, t4 * 512:(t4 + 1) * 512]

        def packw(W):
            # e-major: [e-chunk e][all 256 channels] at cols 256e (v path)
            wT = np.ascontiguousarray(W[cs, :].T).astype(bf)  # [E, 256]
            wp = np.empty((128, 2048), dtype=bf)
            for e in range(8):
                wp[:, e * OC:(e + 1) * OC] = wT[e * 128:(e + 1) * 128, :]
            return wp

        def packw_pair(W):
            # pair-major: pair p at cols 1024p, e-chunk e at 128e within
            wT = np.ascontiguousarray(W[cs, :].T).astype(bf)  # [E, 256]
            wp = np.empty((128, 2048), dtype=bf)
            for p in range(2):
                for e in range(8):
                    wp[:, 1024 * p + 128 * e:1024 * p + 128 * (e + 1)] = \
                        wT[e * 128:(e + 1) * 128, 128 * p:128 * (p + 1)]
            return wp

        in_maps.append({
            "hT": hp,
            "wqT": packw_pair(Wq),
            "wkT": packw_pair(Wk),
            "wvT": packw(Wv),
            "bqp": np.ascontiguousarray(bq[cs].reshape(2, 128).T),
            "bkp": np.ascontiguousarray(bk[cs].reshape(2, 128).T),
            "bvf": np.ascontiguousarray(bv[cs]),
            "mask_t": np.ascontiguousarray(
                (attention_mask[b, 0, 0, :] - SHIFT).reshape(NT, 128).T),
        })
    return in_maps


def kernel(hidden_states, attention_mask, Wq, bq, Wk, bk, Wv, bv):
    in_maps = make_in_maps(hidden_states, attention_mask,
                           Wq, bq, Wk, bk, Wv, bv)
    nc = _get_nc()
    res = run_bass_kernel_spmd(nc, in_maps, list(range(NCORES)))

    full = np.empty((B, S, H * D), dtype=np.float32)
    for c in range(NCORES):
        b, g = divmod(c, 4)
        r = res.results[c]["out"]  # [260, S] unnormalized out^T + denom rows
        for p in range(NPAIR):
            for h in range(2):
                blk = r[130 * p + 65 * h: 130 * p + 65 * h + 65]
                c0 = OC * g + 128 * p + 64 * h
                full[b, :, c0:c0 + 64] = (blk[0:64] / blk[64:65]).T
    return full
